# revision 47
# baseline (speedup 1.0000x reference)
"""APPNP message-passing layer on 8 TRN2 NeuronCores (Bass SPMD kernel).

Math:
  h0 = x @ W.T + b
  deg[v] = in_degree(v) + 1 (self loop), dinv = deg^-1/2
  hop:  h <- (1-a) * Ahat h + a * h0,   Ahat = D^-1/2 (A+I) D^-1/2
  Factored with s = dinv * h:
    A_k[v] = sum_{(u,v) in E} s_k[u] + s_k[v]
    s_{k+1} = (1-a)*dinv^2 * A_k + a*dinv*h0  = d2*A_k + s0a
    out     = (1-a)*dinv   * A_{K-1} + a*h0   = e1*A + s0a*sq

Sharding: nodes block-partitioned across 8 cores (12500/core, padded to
12544 = 128*98).  Edges partitioned by destination core.  Per hop: AllGather
of s (6.4MB/core), dma_gather of source rows (int16 indices -> 4 windows of
25088 rows over the gathered buffer), dma_scatter_add into the local
aggregate.  Node storage uses "partition-major" row order pm = (n%128)*98 +
n//128 so all tile DMAs are 128 contiguous runs.

Semaphore discipline: DMA completions are out-of-order, so each DMA
semaphore only ever has one "batch" in flight: per-parity semaphores for
double-buffered streams (xg/ag/snb-out), a constants semaphore (one batch),
and a gather/scatter semaphore that is waited to its full count after every
issue.  Compute-engine semaphores (t/pv/v/cc) increment in order and use
cumulative thresholds.
"""

import os
import sys

import numpy as np

for _p in ("/opt/trn_rl_repo",):
    if _p not in sys.path and os.path.isdir(_p):
        sys.path.insert(0, _p)

N = 100000
E = 600000
H = 128
ALPHA = 0.1
K_HOPS = 10

NC = 8  # cores
P = 128  # partitions
NPC = 12500  # real nodes per core
T = 98  # node tiles per core
NPAD = P * T  # 12544 padded nodes per core
CHUNK = 2 * NPAD  # 25088 gather window rows (int16-addressable)
NCHUNK = 4
TRASH = 128  # trash rows appended to agg for dummy scatter targets
GB = 1024  # gather/scatter batch (SWDGE ring holds <2048 descriptors/call)
GRP = 16  # node tiles per tail group

GROUPS = [(t0, min(GRP, T - t0)) for t0 in range(0, T, GRP)]

# timing-experiment knobs (wrong results when enabled!)
SKIP_SCATTER = False
NO_BARRIER = False
GQ = 3  # number of SWDGE queues used for gathers
NQUEUES = 4  # total SWDGE queues (scatters use the last one)

LAST_RESULT = None  # test harness reads exec_time_ns from here
LAST_RUN_NS = None  # wall time of the device run (includes compile on 1st call)


def _wrap_idx(vals: np.ndarray) -> np.ndarray:
    """[L] int -> [128, L//16] int16 'wrapped in 16 partitions, replicated 8x'."""
    L = vals.shape[0]
    assert L % 16 == 0
    a = vals.reshape(L // 16, 16).T.astype(np.int16)
    return np.ascontiguousarray(np.tile(a, (8, 1)))


def _pm(n):
    """local node id -> partition-major row id."""
    return (n % P) * T + n // P


def _build_graph(calls: list[tuple[int, int, int]], idxw: int):
    from concourse import bacc, bass, mybir

    f32 = mybir.dt.float32
    i16 = mybir.dt.int16
    mult = mybir.AluOpType.mult
    add = mybir.AluOpType.add

    nc = bacc.Bacc(
        "TRN2", target_bir_lowering=False, debug=False, num_swdge_queues=NQUEUES
    )

    xT = nc.declare_dram_parameter("xT", [P, NPAD], f32, isOutput=False)
    WT = nc.declare_dram_parameter("WT", [P, H], f32, isOutput=False)
    bbp = nc.declare_dram_parameter("bb", [P, H], f32, isOutput=False)
    coef = nc.declare_dram_parameter("coef", [P, 4 * T], f32, isOutput=False)
    gidx = nc.declare_dram_parameter("gidx", [P, idxw], i16, isOutput=False)
    sidx = nc.declare_dram_parameter("sidx", [P, idxw], i16, isOutput=False)
    out_ext = nc.declare_dram_parameter("out", [NPAD, H], f32, isOutput=True)

    s_bounce = nc.dram_tensor("s_bounce", [NPAD, H], f32)
    s_full = nc.dram_tensor("s_full", [NC * NPAD, H], f32, addr_space="Shared")
    agg = nc.dram_tensor("agg", [NPAD + TRASH, H], f32)

    GBB = GB // P  # gather buffer blocks per buf
    NG = len(GROUPS)

    from contextlib import ExitStack

    with ExitStack() as ctx:
        MSLOT = 8  # gather-buffer ring slots
        DEPTH = 5  # gather prefetch depth (< MSLOT)
        xg = ctx.enter_context(nc.sbuf_tensor("xg", [P, 2 * GRP * H], f32))
        s0a = ctx.enter_context(nc.sbuf_tensor("s0a", [P, T * H], f32))
        gbuf = ctx.enter_context(nc.sbuf_tensor("gbuf", [P, MSLOT * GBB * H], f32))
        agb = ctx.enter_context(nc.sbuf_tensor("agb", [P, 2 * GRP * H], f32))
        snb = ctx.enter_context(nc.sbuf_tensor("snb", [P, 2 * GRP * H], f32))
        gix = ctx.enter_context(nc.sbuf_tensor("gix", [P, idxw], i16))
        six = ctx.enter_context(nc.sbuf_tensor("six", [P, idxw], i16))
        cf = ctx.enter_context(nc.sbuf_tensor("cf", [P, 4 * T], f32))
        wt = ctx.enter_context(nc.sbuf_tensor("wt", [P, H], f32))
        bbs = ctx.enter_context(nc.sbuf_tensor("bbs", [P, H], f32))
        tmp = ctx.enter_context(nc.sbuf_tensor("tmp", [P, H], f32))
        ps0 = ctx.enter_context(nc.psum_tensor("ps0", [P, H], f32))
        ps1 = ctx.enter_context(nc.psum_tensor("ps1", [P, H], f32))
        cst_sem = ctx.enter_context(nc.semaphore("cst_sem"))
        xg0_sem = ctx.enter_context(nc.semaphore("xg0_sem"))
        xg1_sem = ctx.enter_context(nc.semaphore("xg1_sem"))
        so0_sem = ctx.enter_context(nc.semaphore("so0_sem"))
        so1_sem = ctx.enter_context(nc.semaphore("so1_sem"))
        agb0_sem = ctx.enter_context(nc.semaphore("agb0_sem"))
        agb1_sem = ctx.enter_context(nc.semaphore("agb1_sem"))
        g_sems = [
            ctx.enter_context(nc.semaphore(f"g{s}_sem")) for s in range(MSLOT)
        ]
        s_sems = [
            ctx.enter_context(nc.semaphore(f"s{s}_sem")) for s in range(MSLOT)
        ]
        cc_sem = ctx.enter_context(nc.semaphore("cc_sem"))
        t_sem = ctx.enter_context(nc.semaphore("t_sem"))
        pv_sem = ctx.enter_context(nc.semaphore("pv_sem"))
        v_sem = ctx.enter_context(nc.semaphore("v_sem"))
        vv_sem = ctx.enter_context(nc.semaphore("vv_sem"))
        ms_sem = ctx.enter_context(nc.semaphore("ms_sem"))
        ps = [ps0, ps1]
        xg_sem = [xg0_sem, xg1_sem]
        so_sem = [so0_sem, so1_sem]
        ag_sem = [agb0_sem, agb1_sem]

        # python-side counters
        st = {
            "xg": [0, 0],
            "so": [0, 0],
            "ag": [0, 0],
            "dio": 0,
            "cc": 0,
            "v": 0,
        }
        cum_tiles = []
        _c = 0
        for _, gn in GROUPS:
            _c += gn
            cum_tiles.append(_c)

        # recorded wait targets
        rec = {
            "xg": [0] * NG,
            "so_init": [0] * NG,
            "so": [[0] * NG for _ in range(K_HOPS)],
            "ag": [[0] * NG for _ in range(K_HOPS)],
        }

        def pm_group_ap(dram, t0, gn):
            # strided AP over [P*T(+pad), H] dram covering PM rows of node
            # tiles [t0, t0+gn): element (p, tt, f) -> row p*T + t0 + tt
            return bass.AP(dram, t0 * H, [[T * H, P], [H, gn], [1, H]])

        def sb_group_ap(sb, buf, gn):
            return bass.AP(sb, buf * GRP * H, [[2 * GRP * H, P], [H, gn], [1, H]])

        def gb_ap(buf, nblk):
            return bass.AP(
                gbuf, buf * GBB * H, [[MSLOT * GBB * H, P], [H, nblk], [1, H]]
            )

        def last_writer(g):
            return NG - 1 if (NG - 1) % 2 == g % 2 else NG - 2

        with nc.Block() as block:

            @block.gpsimd
            def _(gp: bass.BassGpSimd):
                from concourse import library_config

                gp.load_library(library_config.mlp)
                # one register per distinct batch size (to_reg per call would
                # exhaust the register pool)
                nb_vals = sorted({nb for (_, _, nb, _) in calls})
                nb_regs = {}
                for val in nb_vals:
                    r = ctx.enter_context(gp.register(f"nb{val}"))
                    gp.reg_mov(r, val)
                    nb_regs[val] = r
                # zero gather buffer once (partial calls leave unwritten slots)
                gp.memset(gbuf[:, :], 0.0).then_inc(ms_sem, 1)
                gp.wait_ge(ms_sem, 1)
                # per-slot use counters for the gather-buffer ring
                slot_g = [0] * MSLOT
                slot_s = [0] * MSLOT
                # constants (one batch on cst_sem; no further cst increments)
                for dst, srcp in (
                    (wt, WT),
                    (bbs, bbp),
                    (cf, coef),
                    (gix, gidx),
                    (six, sidx),
                ):
                    gp.dma_start(out=dst[:, :], in_=srcp[:, :]).then_inc(cst_sem, 16)
                gp.wait_ge(cst_sem, 80)

                # ---- init: stream xT groups; write s0 out per group
                def xg_load(g):
                    t0, gn = GROUPS[g]
                    par = g % 2
                    gp.dma_start(
                        out=bass.AP(
                            xg, par * GRP * H, [[2 * GRP * H, P], [1, gn * H]]
                        ),
                        in_=xT[:, t0 * H : (t0 + gn) * H],
                    ).then_inc(xg_sem[par], 16)
                    st["xg"][par] += 16
                    rec["xg"][g] = st["xg"][par]

                xg_load(0)
                if NG > 1:
                    xg_load(1)
                for g, (t0, gn) in enumerate(GROUPS):
                    par = g % 2
                    # vector drained through group g (snb writes retired)
                    st["v"] += 1
                    gp.wait_ge(v_sem, st["v"])
                    gp.dma_start(
                        out=pm_group_ap(s_bounce, t0, gn),
                        in_=sb_group_ap(snb, par, gn),
                    ).then_inc(so_sem[par], 16)
                    gp.dma_start(
                        out=pm_group_ap(agg, t0, gn),
                        in_=sb_group_ap(snb, par, gn),
                    ).then_inc(so_sem[par], 16)
                    st["so"][par] += 32
                    rec["so_init"][g] = st["so"][par]
                    if g + 2 < NG:
                        xg_load(g + 2)

                # ---- hops
                for k in range(K_HOPS):
                    # all snb-out DMA (s_bounce/agg writes) must land
                    gp.wait_ge(so_sem[0], st["so"][0])
                    gp.wait_ge(so_sem[1], st["so"][1])
                    gp.collective_compute(
                        "AllGather",
                        mybir.AluOpType.bypass,
                        replica_groups=[list(range(NC))],
                        ins=[s_bounce[:, :]],
                        outs=[s_full[:, :]],
                    ).then_inc(cc_sem)
                    st["cc"] = st.get("cc", 0) + 1
                    gp.wait_ge(cc_sem, st["cc"])

                    # gather / scatter-add, pipelined over a MSLOT-slot ring.
                    # gathers on queue 0, scatters on queue 1; wave barriers
                    # serialize scatters whose dst sets may overlap.
                    def issue_gather(j):
                        cj, o16j, nbj, _ = calls[j]
                        sj = j % MSLOT
                        if SKIP_SCATTER:
                            if slot_g[sj] > 0:
                                gp.wait_ge(g_sems[sj], 16 * slot_g[sj])
                        elif slot_s[sj] > 0:
                            gp.wait_ge(s_sems[sj], 16 * slot_s[sj])
                        gp.dma_gather(
                            out_ap=gb_ap(sj, (nbj + P - 1) // P),
                            in_ap=bass.AP(
                                s_full, cj * CHUNK * H, [[H, CHUNK], [1, H]]
                            ),
                            idxs_ap=gix[:, o16j : o16j + nbj // 16],
                            num_idxs=nbj,
                            num_idxs_reg=nb_regs[nbj],
                            elem_size=H,
                            queue_num=sj % GQ,
                        ).then_inc(g_sems[sj], 16)
                        slot_g[sj] += 1

                    for j in range(min(DEPTH, len(calls))):
                        issue_gather(j)
                    for i, (c, o16, nb, new_wave) in enumerate(calls):
                        s = i % MSLOT
                        if SKIP_SCATTER:
                            if i + DEPTH < len(calls):
                                issue_gather(i + DEPTH)
                            continue
                        if new_wave and not NO_BARRIER:
                            for s2 in range(MSLOT):
                                if slot_s[s2] > 0:
                                    gp.wait_ge(s_sems[s2], 16 * slot_s[s2])
                        gp.wait_ge(g_sems[s], 16 * slot_g[s])
                        gp.dma_scatter_add(
                            out_ap=agg[:, :],
                            in_ap=gb_ap(s, (nb + P - 1) // P),
                            idxs_ap=six[:, o16 : o16 + nb // 16],
                            num_idxs=nb,
                            num_idxs_reg=nb_regs[nb],
                            elem_size=H,
                            queue_num=NQUEUES - 1,
                        ).then_inc(s_sems[s], 16)
                        slot_s[s] += 1
                        if i + DEPTH < len(calls):
                            issue_gather(i + DEPTH)
                    # drain all scatters before the tail reads agg
                    for s2 in range(MSLOT):
                        if slot_s[s2] > 0:
                            gp.wait_ge(s_sems[s2], 16 * slot_s[s2])
                    if SKIP_SCATTER:
                        for s2 in range(MSLOT):
                            if slot_g[s2] > 0:
                                gp.wait_ge(g_sems[s2], 16 * slot_g[s2])

                    # ---- tail: s_{k+1} = d2*agg + s0a  (or final output)
                    def ag_load(g):
                        t0, gn = GROUPS[g]
                        par = g % 2
                        gp.dma_start(
                            out=sb_group_ap(agb, par, gn),
                            in_=pm_group_ap(agg, t0, gn),
                        ).then_inc(ag_sem[par], 16)
                        st["ag"][par] += 16
                        rec["ag"][k][g] = st["ag"][par]

                    ag_load(0)
                    if NG > 1:
                        ag_load(1)
                    for g, (t0, gn) in enumerate(GROUPS):
                        par = g % 2
                        st["v"] += 1
                        gp.wait_ge(v_sem, st["v"])
                        if k < K_HOPS - 1:
                            gp.dma_start(
                                out=pm_group_ap(s_bounce, t0, gn),
                                in_=sb_group_ap(snb, par, gn),
                            ).then_inc(so_sem[par], 16)
                            gp.dma_start(
                                out=pm_group_ap(agg, t0, gn),
                                in_=sb_group_ap(snb, par, gn),
                            ).then_inc(so_sem[par], 16)
                            st["so"][par] += 32
                        else:
                            gp.dma_start(
                                out=pm_group_ap(out_ext, t0, gn),
                                in_=sb_group_ap(snb, par, gn),
                            ).then_inc(so_sem[par], 16)
                            st["so"][par] += 16
                        rec["so"][k][g] = st["so"][par]
                        if g + 2 < NG:
                            ag_load(g + 2)

                gp.wait_ge(so_sem[0], st["so"][0])
                gp.wait_ge(so_sem[1], st["so"][1])

            @block.tensor
            def _(te):
                # self-serialized: wait for own previous matmul before issuing
                te.wait_ge(cst_sem, 80)
                i = 0
                for g, (t0, gn) in enumerate(GROUPS):
                    te.wait_ge(xg_sem[g % 2], rec["xg"][g])
                    for tt in range(gn):
                        if i >= 1:
                            te.wait_ge(t_sem, i)
                        if i >= 2:
                            te.wait_ge(pv_sem, i - 1)
                        te.matmul(
                            ps[i % 2][:, :],
                            bass.AP(
                                xg,
                                (g % 2) * GRP * H + tt * H,
                                [[2 * GRP * H, P], [1, H]],
                            ),
                            wt[:, :],
                        ).then_inc(t_sem)
                        i += 1

            @block.vector
            def _(ve):
                # vv_sem: vector self-sync.  Every op incs vv; "wait vv >= vvc"
                # drains all previously issued vector ops.
                vs = {"vv": 0}

                def vwait(ve_):
                    ve_.wait_ge(vv_sem, vs["vv"])

                def vinc(ins):
                    ins.then_inc(vv_sem)
                    vs["vv"] += 1
                    return ins

                ve.wait_ge(cst_sem, 80)
                # ---- init: consume psum tiles -> s0 (snb) + s0a
                i = 0
                for g, (t0, gn) in enumerate(GROUPS):
                    if g >= 2:
                        ve.wait_ge(so_sem[g % 2], rec["so_init"][g - 2])
                    for tt in range(gn):
                        t = t0 + tt
                        ve.wait_ge(t_sem, i + 1)
                        if i > 0:
                            ve.wait_ge(pv_sem, i)  # prior tile's s0a (tmp reader)
                        vinc(ve.tensor_add(tmp[:, :], ps[i % 2][:, :], bbs[:, :]))
                        vwait(ve)  # tmp written
                        sl = slice(
                            ((g % 2) * GRP + tt) * H, ((g % 2) * GRP + tt + 1) * H
                        )
                        vinc(
                            ve.tensor_scalar_mul(
                                snb[:, sl], tmp[:, :], cf[:, t : t + 1]
                            )
                        )
                        vwait(ve)  # snb write retired before s0a issues
                        ve.tensor_scalar(
                            s0a[:, t * H : (t + 1) * H],
                            tmp[:, :],
                            cf[:, t : t + 1],
                            ALPHA,
                            mult,
                            mult,
                        ).then_inc(pv_sem)
                        if tt == gn - 1:
                            # drain s0a, then signal group done
                            ve.wait_ge(pv_sem, i + 1)
                            ve.sem_inc(v_sem, 1)
                        i += 1

                # all init s0a writes retired before tails read s0a
                ve.wait_ge(pv_sem, cum_tiles[-1])

                # ---- hop tails
                for k in range(K_HOPS):
                    for g, (t0, gn) in enumerate(GROUPS):
                        ve.wait_ge(ag_sem[g % 2], rec["ag"][k][g])
                        # snb[g%2] must be drained by its previous out-DMA
                        if g >= 2:
                            ve.wait_ge(so_sem[g % 2], rec["so"][k][g - 2])
                        else:
                            lw = last_writer(g) if NG > 1 else 0
                            if k > 0:
                                ve.wait_ge(so_sem[g % 2], rec["so"][k - 1][lw])
                            else:
                                ve.wait_ge(so_sem[g % 2], rec["so_init"][lw])
                        for tt in range(gn):
                            t = t0 + tt
                            sl = slice(
                                ((g % 2) * GRP + tt) * H,
                                ((g % 2) * GRP + tt + 1) * H,
                            )
                            if k < K_HOPS - 1:
                                vinc(
                                    ve.scalar_tensor_tensor(
                                        snb[:, sl],
                                        agb[:, sl],
                                        cf[:, T + t : T + t + 1],
                                        s0a[:, t * H : (t + 1) * H],
                                        mult,
                                        add,
                                    )
                                )
                            else:
                                vwait(ve)  # tmp free (prior stt retired)
                                vinc(
                                    ve.tensor_scalar_mul(
                                        tmp[:, :],
                                        s0a[:, t * H : (t + 1) * H],
                                        cf[:, 3 * T + t : 3 * T + t + 1],
                                    )
                                )
                                vwait(ve)  # tmp written
                                vinc(
                                    ve.scalar_tensor_tensor(
                                        snb[:, sl],
                                        agb[:, sl],
                                        cf[:, 2 * T + t : 2 * T + t + 1],
                                        tmp[:, :],
                                        mult,
                                        add,
                                    )
                                )
                        # drain group, then signal
                        vwait(ve)
                        ve.sem_inc(v_sem, 1)

    return nc


def kernel(x, edge_index, W, b):
    global LAST_RESULT
    x = np.asarray(x, dtype=np.float32)
    ei = np.asarray(edge_index).astype(np.int64)
    W = np.asarray(W, dtype=np.float32)
    b = np.asarray(b, dtype=np.float32)

    src, dst = ei[0], ei[1]
    deg = (np.bincount(dst, minlength=N) + 1).astype(np.float32)
    dinv = 1.0 / np.sqrt(deg)

    # --- per-node index maps
    src_core, src_n = src // NPC, src % NPC
    dst_core, dst_n = dst // NPC, dst % NPC
    src_R = src_core * NPAD + _pm(src_n)  # row in s_full
    dst_pm = _pm(dst_n)  # row in agg
    chunk = src_R // CHUNK

    # --- per-(core, wave, chunk) edge lists.
    # dma_scatter_add races on duplicate dst indices among concurrently
    # in-flight calls, so wave w holds the w-th occurrence of each dst within
    # the core's whole edge list; calls within one wave are duplicate-free
    # and may overlap, with a barrier between waves.
    per = {}  # (core, w, chunk) -> (gidx_arr, sidx_arr)
    nwave_max = 0
    cnts = {}
    for core in range(NC):
        m = dst_core == core
        ch = chunk[m]
        gv = (src_R[m] - ch * CHUNK).astype(np.int64)
        sv = dst_pm[m].astype(np.int64)
        # occurrence rank of each edge within its dst group (whole core list)
        o2 = np.argsort(sv, kind="stable")
        ch, gv, sv = ch[o2], gv[o2], sv[o2]
        if sv.shape[0]:
            first = np.r_[True, sv[1:] != sv[:-1]]
            starts = np.where(first)[0]
            rank = np.arange(sv.shape[0]) - starts[np.cumsum(first) - 1]
        else:
            rank = np.zeros(0, dtype=np.int64)
        # order by (rank, chunk)
        o3 = np.lexsort((ch, rank))
        ch, gv, sv, rank = ch[o3], gv[o3], sv[o3], rank[o3]
        nw = int(rank.max()) + 1 if rank.shape[0] else 0
        nwave_max = max(nwave_max, nw)
        for w in range(nw):
            mw = rank == w
            chw, gvw, svw = ch[mw], gv[mw], sv[mw]
            bnd = np.searchsorted(chw, np.arange(NCHUNK + 1))
            for c in range(NCHUNK):
                sl = slice(bnd[c], bnd[c + 1])
                per[(core, w, c)] = (gvw[sl], svw[sl])
                cnts[(core, w, c)] = bnd[c + 1] - bnd[c]

    # global per-(wave, chunk) padded sizes (SPMD: identical across cores)
    cell_sz = {}
    for w in range(nwave_max):
        for c in range(NCHUNK):
            mx = max(cnts.get((core, w, c), 0) for core in range(NC))
            if mx > 0:
                cell_sz[(w, c)] = int(-(-mx // 16) * 16)

    # call layout: (chunk, idx16_offset, n_idxs, new_wave)
    calls = []
    off16 = 0
    for w in range(nwave_max):
        first_of_wave = True
        for c in range(NCHUNK):
            g = cell_sz.get((w, c), 0)
            if g == 0:
                continue
            j = 0
            while j < g:
                nb = min(GB, g - j)
                calls.append((c, off16 + j // 16, nb, first_of_wave and w > 0))
                first_of_wave = False
                j += nb
            off16 += g // 16
    idxw = off16

    # chunk-local row that is always zero in s_full (core 2c's dead node
    # 12500 lives in chunk c at local offset pm(12500))
    ZROW = _pm(NPC)

    # --- build padded index planes per core
    in_maps = []
    for core in range(NC):
        gflat = np.zeros(idxw * 16, dtype=np.int64)
        sflat = np.zeros(idxw * 16, dtype=np.int64)
        pos = 0
        for w in range(nwave_max):
            for c in range(NCHUNK):
                g = cell_sz.get((w, c), 0)
                if g == 0:
                    continue
                gl, slv = per.get(
                    (core, w, c), (np.zeros(0, np.int64), np.zeros(0, np.int64))
                )
                n = gl.shape[0]
                gflat[pos : pos + n] = gl
                sflat[pos : pos + n] = slv
                if g > n:
                    gflat[pos + n : pos + g] = ZROW
                    sflat[pos + n : pos + g] = NPAD + (np.arange(g - n) % TRASH)
                pos += g
        gidx_all = _wrap_idx(gflat)
        sidx_all = _wrap_idx(sflat)

        nodes = slice(core * NPC, (core + 1) * NPC)
        xpad = np.zeros((NPAD, H), dtype=np.float32)
        xpad[:NPC] = x[nodes]
        xTc = np.ascontiguousarray(xpad.T)

        dv = np.zeros(NPAD, dtype=np.float32)
        dv[:NPC] = dinv[nodes]
        sq = np.zeros(NPAD, dtype=np.float32)
        sq[:NPC] = np.sqrt(deg[nodes])

        def to_pt(v):  # [NPAD] -> [P, T] with [p, t] = v[t*P + p]
            return np.ascontiguousarray(v.reshape(T, P).T)

        coef = np.concatenate(
            [
                to_pt(dv),  # dinv
                to_pt((1.0 - ALPHA) * dv * dv),  # d2
                to_pt((1.0 - ALPHA) * dv),  # e1
                to_pt(sq),  # sq
            ],
            axis=1,
        ).astype(np.float32)

        in_maps.append(
            {
                "xT": xTc,
                "WT": np.ascontiguousarray(W.T),
                "bb": np.ascontiguousarray(np.tile(b[None, :], (P, 1))),
                "coef": coef,
                "gidx": gidx_all,
                "sidx": sidx_all,
            }
        )

    nc = _build_graph(calls, idxw)
    nc.compile()

    from concourse.bass_utils import run_bass_kernel_spmd

    import time as _time

    global LAST_RUN_NS
    _t0 = _time.time()
    res = run_bass_kernel_spmd(
        nc,
        in_maps,
        core_ids=list(range(NC)),
        trace=bool(os.environ.get("BASS_TRACE")),
    )
    LAST_RUN_NS = int((_time.time() - _t0) * 1e9)
    LAST_RESULT = res

    out = np.empty((N, H), dtype=np.float32)
    pm_rows = _pm(np.arange(NPC))
    for core in range(NC):
        out[core * NPC : (core + 1) * NPC] = res.results[core]["out"][pm_rows]
    return out


# revision 48
# speedup vs baseline: 7.7340x; 7.7340x over previous
"""APPNP message-passing layer on 8 TRN2 NeuronCores (Bass SPMD kernel).

Math:
  h0 = x @ W.T + b
  deg[v] = in_degree(v) + 1 (self loop), dinv = deg^-1/2
  hop:  h <- (1-a) * Ahat h + a * h0,   Ahat = D^-1/2 (A+I) D^-1/2
  Factored with s = dinv * h:
    A_k[v] = sum_{(u,v) in E} s_k[u] + s_k[v]
    s_{k+1} = (1-a)*dinv^2 * A_k + a*dinv*h0  = d2*A_k + s0a
    out     = (1-a)*dinv   * A_{K-1} + a*h0   = e1*A + s0a*sq

Sharding: nodes block-partitioned across 8 cores (12500/core, padded to
12544 = 128*98).  Edges partitioned by destination core.  Per hop: AllGather
of s (6.4MB/core), dma_gather of source rows (int16 indices -> 4 windows of
25088 rows over the gathered buffer), dma_scatter_add into the local
aggregate.  Node storage uses "partition-major" row order pm = (n%128)*98 +
n//128 so all tile DMAs are 128 contiguous runs.

Semaphore discipline: DMA completions are out-of-order, so each DMA
semaphore only ever has one "batch" in flight: per-parity semaphores for
double-buffered streams (xg/ag/snb-out), a constants semaphore (one batch),
and a gather/scatter semaphore that is waited to its full count after every
issue.  Compute-engine semaphores (t/pv/v/cc) increment in order and use
cumulative thresholds.
"""

import os
import sys

import numpy as np

for _p in ("/opt/trn_rl_repo",):
    if _p not in sys.path and os.path.isdir(_p):
        sys.path.insert(0, _p)

N = 100000
E = 600000
H = 128
ALPHA = 0.1
K_HOPS = 10

NC = 8  # cores
P = 128  # partitions
NPC = 12500  # real nodes per core
T = 98  # node tiles per core
NPAD = P * T  # 12544 padded nodes per core
CHUNK = 2 * NPAD  # 25088 gather window rows (int16-addressable)
NCHUNK = 4
TRASH = 128  # trash rows appended to agg for dummy scatter targets
GB = 1024  # gather/scatter batch (SWDGE ring holds <2048 descriptors/call)
GRP = 16  # node tiles per tail group

GROUPS = [(t0, min(GRP, T - t0)) for t0 in range(0, T, GRP)]

# timing-experiment knobs (wrong results when enabled!)
SKIP_SCATTER = False
NO_BARRIER = False
GQ = 3  # number of SWDGE queues used for gathers
NQUEUES = 4  # total SWDGE queues (scatters use the last one)

LAST_RESULT = None  # test harness reads exec_time_ns from here
LAST_RUN_NS = None  # wall time of the device run (includes compile on 1st call)


def _wrap_idx(vals: np.ndarray) -> np.ndarray:
    """[L] int -> [128, L//16] int16 'wrapped in 16 partitions, replicated 8x'."""
    L = vals.shape[0]
    assert L % 16 == 0
    a = vals.reshape(L // 16, 16).T.astype(np.int16)
    return np.ascontiguousarray(np.tile(a, (8, 1)))


def _pm(n):
    """local node id -> partition-major row id."""
    return (n % P) * T + n // P


def _build_graph(calls: list[tuple[int, int, int]], idxw: int):
    from concourse import bacc, bass, mybir

    f32 = mybir.dt.float32
    i16 = mybir.dt.int16
    mult = mybir.AluOpType.mult
    add = mybir.AluOpType.add

    nc = bacc.Bacc(
        "TRN2", target_bir_lowering=False, debug=False, num_swdge_queues=NQUEUES
    )

    xT = nc.declare_dram_parameter("xT", [P, NPAD], f32, isOutput=False)
    WT = nc.declare_dram_parameter("WT", [P, H], f32, isOutput=False)
    bbp = nc.declare_dram_parameter("bb", [P, H], f32, isOutput=False)
    coef = nc.declare_dram_parameter("coef", [P, 4 * T], f32, isOutput=False)
    gidx = nc.declare_dram_parameter("gidx", [P, idxw], i16, isOutput=False)
    sidx = nc.declare_dram_parameter("sidx", [P, idxw], i16, isOutput=False)
    out_ext = nc.declare_dram_parameter("out", [NPAD, H], f32, isOutput=True)

    s_bounce = nc.dram_tensor("s_bounce", [NPAD, H], f32)
    s_full = nc.dram_tensor("s_full", [NC * NPAD, H], f32, addr_space="Shared")
    agg = nc.dram_tensor("agg", [NPAD + TRASH, H], f32)

    GBB = GB // P  # gather buffer blocks per buf
    NG = len(GROUPS)

    from contextlib import ExitStack

    with ExitStack() as ctx:
        MSLOT = 8  # gather-buffer ring slots
        DEPTH = 5  # gather prefetch depth (< MSLOT)
        xg = ctx.enter_context(nc.sbuf_tensor("xg", [P, 2 * GRP * H], f32))
        s0a = ctx.enter_context(nc.sbuf_tensor("s0a", [P, T * H], f32))
        gbuf = ctx.enter_context(nc.sbuf_tensor("gbuf", [P, MSLOT * GBB * H], f32))
        agb = ctx.enter_context(nc.sbuf_tensor("agb", [P, 2 * GRP * H], f32))
        snb = ctx.enter_context(nc.sbuf_tensor("snb", [P, 2 * GRP * H], f32))
        gix = ctx.enter_context(nc.sbuf_tensor("gix", [P, idxw], i16))
        six = ctx.enter_context(nc.sbuf_tensor("six", [P, idxw], i16))
        cf = ctx.enter_context(nc.sbuf_tensor("cf", [P, 4 * T], f32))
        wt = ctx.enter_context(nc.sbuf_tensor("wt", [P, H], f32))
        bbs = ctx.enter_context(nc.sbuf_tensor("bbs", [P, H], f32))
        tmp = ctx.enter_context(nc.sbuf_tensor("tmp", [P, H], f32))
        ps0 = ctx.enter_context(nc.psum_tensor("ps0", [P, H], f32))
        ps1 = ctx.enter_context(nc.psum_tensor("ps1", [P, H], f32))
        cst_sem = ctx.enter_context(nc.semaphore("cst_sem"))
        xg0_sem = ctx.enter_context(nc.semaphore("xg0_sem"))
        xg1_sem = ctx.enter_context(nc.semaphore("xg1_sem"))
        so0_sem = ctx.enter_context(nc.semaphore("so0_sem"))
        so1_sem = ctx.enter_context(nc.semaphore("so1_sem"))
        agb0_sem = ctx.enter_context(nc.semaphore("agb0_sem"))
        agb1_sem = ctx.enter_context(nc.semaphore("agb1_sem"))
        g_sems = [
            ctx.enter_context(nc.semaphore(f"g{s}_sem")) for s in range(MSLOT)
        ]
        s_sems = [
            ctx.enter_context(nc.semaphore(f"s{s}_sem")) for s in range(MSLOT)
        ]
        cc_sem = ctx.enter_context(nc.semaphore("cc_sem"))
        t_sem = ctx.enter_context(nc.semaphore("t_sem"))
        pv_sem = ctx.enter_context(nc.semaphore("pv_sem"))
        v_sem = ctx.enter_context(nc.semaphore("v_sem"))
        vv_sem = ctx.enter_context(nc.semaphore("vv_sem"))
        ms_sem = ctx.enter_context(nc.semaphore("ms_sem"))
        ps = [ps0, ps1]
        xg_sem = [xg0_sem, xg1_sem]
        so_sem = [so0_sem, so1_sem]
        ag_sem = [agb0_sem, agb1_sem]

        # python-side counters
        st = {
            "xg": [0, 0],
            "so": [0, 0],
            "ag": [0, 0],
            "dio": 0,
            "cc": 0,
            "v": 0,
        }
        cum_tiles = []
        _c = 0
        for _, gn in GROUPS:
            _c += gn
            cum_tiles.append(_c)

        # recorded wait targets
        rec = {
            "xg": [0] * NG,
            "so_init": [0] * NG,
            "so": [[0] * NG for _ in range(K_HOPS)],
            "ag": [[0] * NG for _ in range(K_HOPS)],
        }

        def pm_group_ap(dram, t0, gn):
            # strided AP over [P*T(+pad), H] dram covering PM rows of node
            # tiles [t0, t0+gn): element (p, tt, f) -> row p*T + t0 + tt
            return bass.AP(dram, t0 * H, [[T * H, P], [H, gn], [1, H]])

        def sb_group_ap(sb, buf, gn):
            return bass.AP(sb, buf * GRP * H, [[2 * GRP * H, P], [H, gn], [1, H]])

        def gb_ap(buf, nblk):
            return bass.AP(
                gbuf, buf * GBB * H, [[MSLOT * GBB * H, P], [H, nblk], [1, H]]
            )

        def last_writer(g):
            return NG - 1 if (NG - 1) % 2 == g % 2 else NG - 2

        with nc.Block() as block:

            @block.gpsimd
            def _(gp: bass.BassGpSimd):
                from concourse import library_config

                gp.load_library(library_config.mlp)
                # one register per distinct batch size (to_reg per call would
                # exhaust the register pool)
                nb_vals = sorted({nb for (_, _, nb, _) in calls})
                nb_regs = {}
                for val in nb_vals:
                    r = ctx.enter_context(gp.register(f"nb{val}"))
                    gp.reg_mov(r, val)
                    nb_regs[val] = r
                # zero gather buffer once (partial calls leave unwritten slots)
                gp.memset(gbuf[:, :], 0.0).then_inc(ms_sem, 1)
                gp.wait_ge(ms_sem, 1)
                # per-slot use counters for the gather-buffer ring
                slot_g = [0] * MSLOT
                slot_s = [0] * MSLOT
                # constants (one batch on cst_sem; no further cst increments)
                for dst, srcp in (
                    (wt, WT),
                    (bbs, bbp),
                    (cf, coef),
                    (gix, gidx),
                    (six, sidx),
                ):
                    gp.dma_start(out=dst[:, :], in_=srcp[:, :]).then_inc(cst_sem, 16)
                gp.wait_ge(cst_sem, 80)

                # ---- init: stream xT groups; write s0 out per group
                def xg_load(g):
                    t0, gn = GROUPS[g]
                    par = g % 2
                    gp.dma_start(
                        out=bass.AP(
                            xg, par * GRP * H, [[2 * GRP * H, P], [1, gn * H]]
                        ),
                        in_=xT[:, t0 * H : (t0 + gn) * H],
                    ).then_inc(xg_sem[par], 16)
                    st["xg"][par] += 16
                    rec["xg"][g] = st["xg"][par]

                xg_load(0)
                if NG > 1:
                    xg_load(1)
                for g, (t0, gn) in enumerate(GROUPS):
                    par = g % 2
                    # vector drained through group g (snb writes retired)
                    st["v"] += 1
                    gp.wait_ge(v_sem, st["v"])
                    gp.dma_start(
                        out=pm_group_ap(s_bounce, t0, gn),
                        in_=sb_group_ap(snb, par, gn),
                    ).then_inc(so_sem[par], 16)
                    gp.dma_start(
                        out=pm_group_ap(agg, t0, gn),
                        in_=sb_group_ap(snb, par, gn),
                    ).then_inc(so_sem[par], 16)
                    st["so"][par] += 32
                    rec["so_init"][g] = st["so"][par]
                    if g + 2 < NG:
                        xg_load(g + 2)

                # ---- hops
                for k in range(K_HOPS):
                    # all snb-out DMA (s_bounce/agg writes) must land
                    gp.wait_ge(so_sem[0], st["so"][0])
                    gp.wait_ge(so_sem[1], st["so"][1])
                    gp.collective_compute(
                        "AllGather",
                        mybir.AluOpType.bypass,
                        replica_groups=[list(range(NC))],
                        ins=[s_bounce[:, :]],
                        outs=[s_full[:, :]],
                    ).then_inc(cc_sem)
                    st["cc"] = st.get("cc", 0) + 1
                    gp.wait_ge(cc_sem, st["cc"])

                    # gather / scatter-add, pipelined over a MSLOT-slot ring.
                    # gathers on queue 0, scatters on queue 1; wave barriers
                    # serialize scatters whose dst sets may overlap.
                    def issue_gather(j):
                        cj, o16j, nbj, _ = calls[j]
                        sj = j % MSLOT
                        if SKIP_SCATTER:
                            if slot_g[sj] > 0:
                                gp.wait_ge(g_sems[sj], 16 * slot_g[sj])
                        elif slot_s[sj] > 0:
                            gp.wait_ge(s_sems[sj], 16 * slot_s[sj])
                        gp.dma_gather(
                            out_ap=gb_ap(sj, (nbj + P - 1) // P),
                            in_ap=bass.AP(
                                s_full, cj * CHUNK * H, [[H, CHUNK], [1, H]]
                            ),
                            idxs_ap=gix[:, o16j : o16j + nbj // 16],
                            num_idxs=nbj,
                            num_idxs_reg=nb_regs[nbj],
                            elem_size=H,
                            queue_num=sj % GQ,
                        ).then_inc(g_sems[sj], 16)
                        slot_g[sj] += 1

                    for j in range(min(DEPTH, len(calls))):
                        issue_gather(j)
                    for i, (c, o16, nb, new_wave) in enumerate(calls):
                        s = i % MSLOT
                        # feed gather queues before any scatter-side waits
                        if i + DEPTH < len(calls):
                            issue_gather(i + DEPTH)
                        if SKIP_SCATTER:
                            continue
                        if new_wave and not NO_BARRIER:
                            for s2 in range(MSLOT):
                                if slot_s[s2] > 0:
                                    gp.wait_ge(s_sems[s2], 16 * slot_s[s2])
                        gp.wait_ge(g_sems[s], 16 * slot_g[s])
                        gp.dma_scatter_add(
                            out_ap=agg[:, :],
                            in_ap=gb_ap(s, (nb + P - 1) // P),
                            idxs_ap=six[:, o16 : o16 + nb // 16],
                            num_idxs=nb,
                            num_idxs_reg=nb_regs[nb],
                            elem_size=H,
                            queue_num=NQUEUES - 1,
                        ).then_inc(s_sems[s], 16)
                        slot_s[s] += 1
                    # drain all scatters before the tail reads agg
                    for s2 in range(MSLOT):
                        if slot_s[s2] > 0:
                            gp.wait_ge(s_sems[s2], 16 * slot_s[s2])
                    if SKIP_SCATTER:
                        for s2 in range(MSLOT):
                            if slot_g[s2] > 0:
                                gp.wait_ge(g_sems[s2], 16 * slot_g[s2])

                    # ---- tail: s_{k+1} = d2*agg + s0a  (or final output)
                    def ag_load(g):
                        t0, gn = GROUPS[g]
                        par = g % 2
                        gp.dma_start(
                            out=sb_group_ap(agb, par, gn),
                            in_=pm_group_ap(agg, t0, gn),
                        ).then_inc(ag_sem[par], 16)
                        st["ag"][par] += 16
                        rec["ag"][k][g] = st["ag"][par]

                    ag_load(0)
                    if NG > 1:
                        ag_load(1)
                    for g, (t0, gn) in enumerate(GROUPS):
                        par = g % 2
                        st["v"] += 1
                        gp.wait_ge(v_sem, st["v"])
                        if k < K_HOPS - 1:
                            gp.dma_start(
                                out=pm_group_ap(s_bounce, t0, gn),
                                in_=sb_group_ap(snb, par, gn),
                            ).then_inc(so_sem[par], 16)
                            gp.dma_start(
                                out=pm_group_ap(agg, t0, gn),
                                in_=sb_group_ap(snb, par, gn),
                            ).then_inc(so_sem[par], 16)
                            st["so"][par] += 32
                        else:
                            gp.dma_start(
                                out=pm_group_ap(out_ext, t0, gn),
                                in_=sb_group_ap(snb, par, gn),
                            ).then_inc(so_sem[par], 16)
                            st["so"][par] += 16
                        rec["so"][k][g] = st["so"][par]
                        if g + 2 < NG:
                            ag_load(g + 2)

                gp.wait_ge(so_sem[0], st["so"][0])
                gp.wait_ge(so_sem[1], st["so"][1])

            @block.tensor
            def _(te):
                # self-serialized: wait for own previous matmul before issuing
                te.wait_ge(cst_sem, 80)
                i = 0
                for g, (t0, gn) in enumerate(GROUPS):
                    te.wait_ge(xg_sem[g % 2], rec["xg"][g])
                    for tt in range(gn):
                        if i >= 1:
                            te.wait_ge(t_sem, i)
                        if i >= 2:
                            te.wait_ge(pv_sem, i - 1)
                        te.matmul(
                            ps[i % 2][:, :],
                            bass.AP(
                                xg,
                                (g % 2) * GRP * H + tt * H,
                                [[2 * GRP * H, P], [1, H]],
                            ),
                            wt[:, :],
                        ).then_inc(t_sem)
                        i += 1

            @block.vector
            def _(ve):
                # vv_sem: vector self-sync.  Every op incs vv; "wait vv >= vvc"
                # drains all previously issued vector ops.
                vs = {"vv": 0}

                def vwait(ve_):
                    ve_.wait_ge(vv_sem, vs["vv"])

                def vinc(ins):
                    ins.then_inc(vv_sem)
                    vs["vv"] += 1
                    return ins

                ve.wait_ge(cst_sem, 80)
                # ---- init: consume psum tiles -> s0 (snb) + s0a
                i = 0
                for g, (t0, gn) in enumerate(GROUPS):
                    if g >= 2:
                        ve.wait_ge(so_sem[g % 2], rec["so_init"][g - 2])
                    for tt in range(gn):
                        t = t0 + tt
                        ve.wait_ge(t_sem, i + 1)
                        if i > 0:
                            ve.wait_ge(pv_sem, i)  # prior tile's s0a (tmp reader)
                        vinc(ve.tensor_add(tmp[:, :], ps[i % 2][:, :], bbs[:, :]))
                        vwait(ve)  # tmp written
                        sl = slice(
                            ((g % 2) * GRP + tt) * H, ((g % 2) * GRP + tt + 1) * H
                        )
                        vinc(
                            ve.tensor_scalar_mul(
                                snb[:, sl], tmp[:, :], cf[:, t : t + 1]
                            )
                        )
                        vwait(ve)  # snb write retired before s0a issues
                        ve.tensor_scalar(
                            s0a[:, t * H : (t + 1) * H],
                            tmp[:, :],
                            cf[:, t : t + 1],
                            ALPHA,
                            mult,
                            mult,
                        ).then_inc(pv_sem)
                        if tt == gn - 1:
                            # drain s0a, then signal group done
                            ve.wait_ge(pv_sem, i + 1)
                            ve.sem_inc(v_sem, 1)
                        i += 1

                # all init s0a writes retired before tails read s0a
                ve.wait_ge(pv_sem, cum_tiles[-1])

                # ---- hop tails
                for k in range(K_HOPS):
                    for g, (t0, gn) in enumerate(GROUPS):
                        ve.wait_ge(ag_sem[g % 2], rec["ag"][k][g])
                        # snb[g%2] must be drained by its previous out-DMA
                        if g >= 2:
                            ve.wait_ge(so_sem[g % 2], rec["so"][k][g - 2])
                        else:
                            lw = last_writer(g) if NG > 1 else 0
                            if k > 0:
                                ve.wait_ge(so_sem[g % 2], rec["so"][k - 1][lw])
                            else:
                                ve.wait_ge(so_sem[g % 2], rec["so_init"][lw])
                        for tt in range(gn):
                            t = t0 + tt
                            sl = slice(
                                ((g % 2) * GRP + tt) * H,
                                ((g % 2) * GRP + tt + 1) * H,
                            )
                            if k < K_HOPS - 1:
                                vinc(
                                    ve.scalar_tensor_tensor(
                                        snb[:, sl],
                                        agb[:, sl],
                                        cf[:, T + t : T + t + 1],
                                        s0a[:, t * H : (t + 1) * H],
                                        mult,
                                        add,
                                    )
                                )
                            else:
                                vwait(ve)  # tmp free (prior stt retired)
                                vinc(
                                    ve.tensor_scalar_mul(
                                        tmp[:, :],
                                        s0a[:, t * H : (t + 1) * H],
                                        cf[:, 3 * T + t : 3 * T + t + 1],
                                    )
                                )
                                vwait(ve)  # tmp written
                                vinc(
                                    ve.scalar_tensor_tensor(
                                        snb[:, sl],
                                        agb[:, sl],
                                        cf[:, 2 * T + t : 2 * T + t + 1],
                                        tmp[:, :],
                                        mult,
                                        add,
                                    )
                                )
                        # drain group, then signal
                        vwait(ve)
                        ve.sem_inc(v_sem, 1)

    return nc


def kernel(x, edge_index, W, b):
    global LAST_RESULT
    x = np.asarray(x, dtype=np.float32)
    ei = np.asarray(edge_index).astype(np.int64)
    W = np.asarray(W, dtype=np.float32)
    b = np.asarray(b, dtype=np.float32)

    src, dst = ei[0], ei[1]
    deg = (np.bincount(dst, minlength=N) + 1).astype(np.float32)
    dinv = 1.0 / np.sqrt(deg)

    # --- per-node index maps
    src_core, src_n = src // NPC, src % NPC
    dst_core, dst_n = dst // NPC, dst % NPC
    src_R = src_core * NPAD + _pm(src_n)  # row in s_full
    dst_pm = _pm(dst_n)  # row in agg
    chunk = src_R // CHUNK

    # --- per-(core, wave, chunk) edge lists.
    # dma_scatter_add races on duplicate dst indices among concurrently
    # in-flight calls, so wave w holds the w-th occurrence of each dst within
    # the core's whole edge list; calls within one wave are duplicate-free
    # and may overlap, with a barrier between waves.
    per = {}  # (core, w, chunk) -> (gidx_arr, sidx_arr)
    nwave_max = 0
    cnts = {}
    for core in range(NC):
        m = dst_core == core
        ch = chunk[m]
        gv = (src_R[m] - ch * CHUNK).astype(np.int64)
        sv = dst_pm[m].astype(np.int64)
        # occurrence rank of each edge within its dst group (whole core list)
        o2 = np.argsort(sv, kind="stable")
        ch, gv, sv = ch[o2], gv[o2], sv[o2]
        if sv.shape[0]:
            first = np.r_[True, sv[1:] != sv[:-1]]
            starts = np.where(first)[0]
            rank = np.arange(sv.shape[0]) - starts[np.cumsum(first) - 1]
        else:
            rank = np.zeros(0, dtype=np.int64)
        # order by (rank, chunk)
        o3 = np.lexsort((ch, rank))
        ch, gv, sv, rank = ch[o3], gv[o3], sv[o3], rank[o3]
        nw = int(rank.max()) + 1 if rank.shape[0] else 0
        nwave_max = max(nwave_max, nw)
        for w in range(nw):
            mw = rank == w
            chw, gvw, svw = ch[mw], gv[mw], sv[mw]
            bnd = np.searchsorted(chw, np.arange(NCHUNK + 1))
            for c in range(NCHUNK):
                sl = slice(bnd[c], bnd[c + 1])
                per[(core, w, c)] = (gvw[sl], svw[sl])
                cnts[(core, w, c)] = bnd[c + 1] - bnd[c]

    # global per-(wave, chunk) padded sizes (SPMD: identical across cores)
    cell_sz = {}
    for w in range(nwave_max):
        for c in range(NCHUNK):
            mx = max(cnts.get((core, w, c), 0) for core in range(NC))
            if mx > 0:
                cell_sz[(w, c)] = int(-(-mx // 16) * 16)

    # call layout: (chunk, idx16_offset, n_idxs, new_wave)
    calls = []
    off16 = 0
    for w in range(nwave_max):
        first_of_wave = True
        for c in range(NCHUNK):
            g = cell_sz.get((w, c), 0)
            if g == 0:
                continue
            j = 0
            while j < g:
                nb = min(GB, g - j)
                calls.append((c, off16 + j // 16, nb, first_of_wave and w > 0))
                first_of_wave = False
                j += nb
            off16 += g // 16
    idxw = off16

    # chunk-local row that is always zero in s_full (core 2c's dead node
    # 12500 lives in chunk c at local offset pm(12500))
    ZROW = _pm(NPC)

    # --- build padded index planes per core
    in_maps = []
    for core in range(NC):
        gflat = np.zeros(idxw * 16, dtype=np.int64)
        sflat = np.zeros(idxw * 16, dtype=np.int64)
        pos = 0
        for w in range(nwave_max):
            for c in range(NCHUNK):
                g = cell_sz.get((w, c), 0)
                if g == 0:
                    continue
                gl, slv = per.get(
                    (core, w, c), (np.zeros(0, np.int64), np.zeros(0, np.int64))
                )
                n = gl.shape[0]
                gflat[pos : pos + n] = gl
                sflat[pos : pos + n] = slv
                if g > n:
                    gflat[pos + n : pos + g] = ZROW
                    sflat[pos + n : pos + g] = NPAD + (np.arange(g - n) % TRASH)
                pos += g
        gidx_all = _wrap_idx(gflat)
        sidx_all = _wrap_idx(sflat)

        nodes = slice(core * NPC, (core + 1) * NPC)
        xpad = np.zeros((NPAD, H), dtype=np.float32)
        xpad[:NPC] = x[nodes]
        xTc = np.ascontiguousarray(xpad.T)

        dv = np.zeros(NPAD, dtype=np.float32)
        dv[:NPC] = dinv[nodes]
        sq = np.zeros(NPAD, dtype=np.float32)
        sq[:NPC] = np.sqrt(deg[nodes])

        def to_pt(v):  # [NPAD] -> [P, T] with [p, t] = v[t*P + p]
            return np.ascontiguousarray(v.reshape(T, P).T)

        coef = np.concatenate(
            [
                to_pt(dv),  # dinv
                to_pt((1.0 - ALPHA) * dv * dv),  # d2
                to_pt((1.0 - ALPHA) * dv),  # e1
                to_pt(sq),  # sq
            ],
            axis=1,
        ).astype(np.float32)

        in_maps.append(
            {
                "xT": xTc,
                "WT": np.ascontiguousarray(W.T),
                "bb": np.ascontiguousarray(np.tile(b[None, :], (P, 1))),
                "coef": coef,
                "gidx": gidx_all,
                "sidx": sidx_all,
            }
        )

    nc = _build_graph(calls, idxw)
    nc.compile()

    from concourse.bass_utils import run_bass_kernel_spmd

    import time as _time

    global LAST_RUN_NS
    _t0 = _time.time()
    res = run_bass_kernel_spmd(
        nc,
        in_maps,
        core_ids=list(range(NC)),
        trace=bool(os.environ.get("BASS_TRACE")),
    )
    LAST_RUN_NS = int((_time.time() - _t0) * 1e9)
    LAST_RESULT = res

    out = np.empty((N, H), dtype=np.float32)
    pm_rows = _pm(np.arange(NPC))
    for core in range(NC):
        out[core * NPC : (core + 1) * NPC] = res.results[core]["out"][pm_rows]
    return out
